# revision 15
# baseline (speedup 1.0000x reference)
"""Single-head causal attention on 8 Trainium2 NeuronCores.

Problem: x[8, 2048, 1024] f32, per-head projections (H=64), causal
softmax attention scaled by C**-0.5.

Strategy: data-parallel over batch (1 batch element per core). Per core
everything is kept in transposed layout so no fp32 on-chip transposes of
large tensors are needed:

  - host pre-casts x to fp16 and lays it out wave-major as
    xw[w][p][c*512+t] so each of the 4 waves is one DMA instruction with
    8KB-contiguous reads per partition (max HBM efficiency); waves are
    issued on 4 different engine queues so issue cost is not serialized
  - Q^T/K^T [64, T] computed with the weight matrices stationary; Q and
    K packed into one PSUM tile via PE column tiling (concurrent on HW)
  - V^T computed the same way; tj pairs column-tiled where both waves
    are resident; V then PE-transposed to [Tk, H] for the PV matmuls
  - S^T tiles [128 Tk, 512 Tq] = (K^T chunk).T @ Q^T, two at a time via
    PE row tiling into one two-bank PSUM tile
  - causal masking without any mask loads: diagonal S blocks are
    column-trimmed in the matmul itself, exp is emitted only over the
    valid column ranges, and the 128x128 triangle at the diagonal is
    zeroed by one small DVE multiply with an SBUF triangle constant
  - softmax runs without max-subtraction (logits are O(1) by
    construction)
  - P@V: V tiles [128 Tk, 65] carry an appended ones column, so the
    softmax denominator falls out of the same PSUM accumulation as the
    numerator; PV accumulation runs diagonal-first so trimmed moving
    ranges never conflict with the accumulation start flag
  - a short chain of dummy matmuls warms the PE clock (HAM ramp) during
    the DMA-bound startup window
  - normalization (divide by denominator row) + final transpose on host

Outputs are returned as float32 [8, 2048, 64].
"""

import numpy as np

import concourse.mybir as mybir
import concourse.tile as tile
from concourse import bacc
from concourse.bass_utils import run_bass_kernel_spmd

B, T, C, H = 8, 2048, 1024, 64
N_CORES = 8
TQ = 512          # Tq chunk (one fp32 PSUM bank)
N_JQ = T // TQ    # 4
N_TK = T // 128   # 16
N_KC = C // 128   # 8  contraction chunks for projections

DT16 = mybir.dt.float16  # fp16: same PE/DVE/DMA speed as bf16, 8 more mantissa bits
F32 = mybir.dt.float32
AF = mybir.ActivationFunctionType

WARM_N = 14       # PE clock pre-warm matmuls during the DMA-bound startup

# column offsets inside the packed fp16 constant tensor
CST_WQK = 0            # [128, 8*128] striped [wq | wk] per c-chunk
CST_WV = 1024          # [128, 8*64] striped wv per c-chunk
CST_ID = 1536          # [128, 64] eye(64) stacked twice
CST_TRI = 1600         # [128, 128] upper triangle (col >= row) 0/1
CST_W = 1728

_CACHED_NC = None


def build_program(reps=1):
    nc = bacc.Bacc("TRN2", target_bir_lowering=False, debug=False,
                   num_devices=N_CORES)

    xw_d = nc.dram_tensor("xw", [N_JQ, 128, N_KC * TQ], DT16,
                          kind="ExternalInput").ap()
    cst_d = nc.dram_tensor("cst", [128, CST_W], DT16,
                           kind="ExternalInput").ap()
    bqk_d = nc.dram_tensor("bqk", [128, 1], F32, kind="ExternalInput").ap()
    bv2_d = nc.dram_tensor("bv2", [128, 1], F32, kind="ExternalInput").ap()
    y_d = nc.dram_tensor("y", [H + 1, T], F32, kind="ExternalOutput").ap()

    with tile.TileContext(nc) as tc:
        with (
            tc.tile_pool(name="sb", bufs=1) as sb,
            tc.tile_pool(name="et", bufs=6) as et_pool,
            tc.tile_pool(name="ysb", bufs=2) as y_pool,
            tc.tile_pool(name="ps_proj", bufs=2, space="PSUM") as ps_proj,
            tc.tile_pool(name="ps_s", bufs=2, space="PSUM") as ps_s,
            tc.tile_pool(name="ps_o", bufs=2, space="PSUM") as ps_o,
        ):
            cst = sb.tile([128, CST_W], DT16, tag="cst")
            bqk_sb = sb.tile([128, 1], F32, tag="bqk")
            bv2_sb = sb.tile([128, 1], F32, tag="bv2")
            xw = [sb.tile([128, N_KC * TQ], DT16, tag=f"xw{w}",
                          name=f"xw{w}")
                  for w in range(N_JQ)]
            q2 = sb.tile([128, T], DT16, tag="q2")
            k2 = sb.tile([128, T], DT16, tag="k2")
            vT = sb.tile([128, T // 2], DT16, tag="vT")
            v_sb = sb.tile([128, N_TK, H + 1], DT16, tag="v")
            warm = sb.tile([128, TQ], DT16, tag="warm")

            def wqk_ap(c, lo, hi):
                return cst[:, CST_WQK + c * 128 + lo:CST_WQK + c * 128 + hi]

            def wv_ap(c):
                return cst[:, CST_WV + c * H:CST_WV + (c + 1) * H]

            def ident_ap(half):
                return cst[64 * half:64 * half + 64, CST_ID:CST_ID + H]

            tri = cst[:, CST_TRI:CST_TRI + 128]

            # ---- DMA issue ----------------------------------------------
            # DMAs on one engine queue serialize on one hardware ring, and
            # ring rate scales with contiguous element size (~45GB/s at
            # 2KB, ~2x at 8KB).  Waves are therefore split by PARTITION
            # half — each piece keeps the full 8KB/partition contiguous
            # run of the wave-major layout — and spread so each queue's
            # FIFO delivers in consumption order.  Tiny bias loads go
            # first so they never sit behind megabytes of x.
            def xh(w, h):
                sl = slice(h * 4 * TQ, (h + 1) * 4 * TQ)
                return xw[w][:, sl], xw_d[w][:, sl]

            nc.sync.dma_start(*xh(0, 0))
            nc.gpsimd.dma_start(bqk_sb[:], bqk_d)
            nc.scalar.dma_start(cst[:, 0:CST_WV], cst_d[:, 0:CST_WV])
            nc.vector.memset(warm[:], 0.5)
            nc.vector.memset(v_sb[:], 1.0)
            nc.gpsimd.dma_start(bv2_sb[:], bv2_d)
            nc.gpsimd.dma_start(*xh(0, 1))
            nc.scalar.dma_start(cst[:, CST_WV:CST_W], cst_d[:, CST_WV:CST_W])
            nc.sync.dma_start(*xh(1, 0))
            nc.gpsimd.dma_start(*xh(1, 1))
            nc.scalar.dma_start(*xh(3, 0))
            nc.sync.dma_start(*xh(2, 0))
            nc.gpsimd.dma_start(*xh(2, 1))
            nc.scalar.dma_start(*xh(3, 1))

            # ---- PE clock warm-up while DMA streams in ------------------
            if WARM_N:
                wps = ps_proj.tile([64, TQ], F32, tag="proj", name="warm_ps")
                for _w in range(WARM_N):
                    nc.tensor.matmul(wps[:], warm[:, 0:64], warm[:],
                                     start=True, stop=True,
                                     tile_position=(0, 0),
                                     skip_group_check=True)

            # ---- projection thunks --------------------------------------
            def qk_thunks(tj):
                sl = slice(tj * TQ, (tj + 1) * TQ)
                st = {}
                th = []

                def qk_mm(c):
                    if c == 0:
                        st["ps"] = ps_proj.tile([128, TQ], F32, tag="proj",
                                                name="ps_qk")
                    ps = st["ps"]
                    nc.tensor.matmul(ps[0:64, :], wqk_ap(c, 0, 64),
                                     xw[tj][:, c * TQ:(c + 1) * TQ],
                                     start=(c == 0), stop=(c == N_KC - 1),
                                     tile_position=(0, 0),
                                     skip_group_check=True)
                    nc.tensor.matmul(ps[64:128, :], wqk_ap(c, 64, 128),
                                     xw[tj][:, c * TQ:(c + 1) * TQ],
                                     start=(c == 0), stop=(c == N_KC - 1),
                                     tile_position=(0, 64),
                                     skip_group_check=True)
                for c in range(N_KC):
                    th.append(lambda c=c: qk_mm(c))

                def qk_epi():
                    ps = st["ps"]
                    nc.vector.tensor_scalar_add(q2[0:64, sl], ps[0:64, :],
                                                bqk_sb[0:64, :])
                    nc.vector.tensor_scalar_add(k2[64:128, sl], ps[64:128, :],
                                                bqk_sb[64:128, :])
                    nc.gpsimd.tensor_copy(q2[64:128, sl], q2[0:64, sl])
                    nc.gpsimd.tensor_copy(k2[0:64, sl], k2[64:128, sl])
                th.append(qk_epi)
                return th

            def v_tr_thunk(tj, st):
                # transpose 4 [64,128] V^T blocks of chunk tj into v_sb
                def tr():
                    half = tj % 2
                    base = (tj // 2) * TQ
                    pt = ps_proj.tile([128, 4, H], DT16, tag="proj",
                                      name=f"pt{tj}")
                    for r in range(4):
                        nc.tensor.transpose(
                            pt[:, r, :],
                            vT[64 * half:64 * half + 64,
                               base + r * 128:base + (r + 1) * 128],
                            ident_ap(half))
                    nc.vector.tensor_copy(v_sb[:, 4 * tj:4 * tj + 4, 0:H],
                                          pt[:, :, :])
                return tr

            def v_single_thunks(tj):
                st = {}
                th = []
                half = tj % 2

                def v_mm(c):
                    if c == 0:
                        st["psv"] = ps_proj.tile([128, TQ], F32, tag="proj",
                                                 name=f"ps_v{tj}")
                    nc.tensor.matmul(st["psv"][64 * half:64 * half + 64, :],
                                     wv_ap(c), xw[tj][:, c * TQ:(c + 1) * TQ],
                                     start=(c == 0), stop=(c == N_KC - 1),
                                     tile_position=(0, 64 * half),
                                     skip_group_check=True)
                for c in range(N_KC):
                    th.append(lambda c=c: v_mm(c))

                def v_epi():
                    nc.vector.tensor_scalar_add(
                        vT[64 * half:64 * half + 64,
                           (tj // 2) * TQ:(tj // 2 + 1) * TQ],
                        st["psv"][64 * half:64 * half + 64, :],
                        bv2_sb[64 * half:64 * half + 64, :])
                th.append(v_epi)
                th.append(v_tr_thunk(tj, st))
                return th

            def v_pair_thunks(tja, tjb):
                # tja even, tjb = tja+1: column-tiled pair, both waves loaded
                st = {}
                th = []

                def v_mm(c):
                    if c == 0:
                        st["psv"] = ps_proj.tile([128, TQ], F32, tag="proj",
                                                 name=f"ps_v{tja}{tjb}")
                    for tj in (tja, tjb):
                        half = tj % 2
                        nc.tensor.matmul(
                            st["psv"][64 * half:64 * half + 64, :], wv_ap(c),
                            xw[tj][:, c * TQ:(c + 1) * TQ],
                            start=(c == 0), stop=(c == N_KC - 1),
                            tile_position=(0, 64 * half),
                            skip_group_check=True)
                for c in range(N_KC):
                    th.append(lambda c=c: v_mm(c))

                def v_epi():
                    for tj in (tja, tjb):
                        half = tj % 2
                        nc.vector.tensor_scalar_add(
                            vT[64 * half:64 * half + 64,
                               (tj // 2) * TQ:(tj // 2 + 1) * TQ],
                            st["psv"][64 * half:64 * half + 64, :],
                            bv2_sb[64 * half:64 * half + 64, :])
                th.append(v_epi)
                th.append(v_tr_thunk(tja, st))
                th.append(v_tr_thunk(tjb, st))
                return th

            # ---- attention thunks ---------------------------------------
            def attn_thunks(jq):
                n_ik = (jq + 1) * 4
                n_pairs = n_ik // 2
                # natural order: the accumulation start flag lands on the
                # full-range s=0 PV; the trimmed diagonal pairs come last
                # (max slack for the freshest V transposes, short tail)
                s_list = list(range(n_pairs))
                st = {}
                th = []

                def s_exp(s, si):
                    if si == 0:
                        st["po"] = ps_o.tile([H + 1, TQ], F32, tag="out",
                                             name="po")
                    ps2 = ps_s.tile([128, 2 * TQ], F32, tag="s")
                    eT = et_pool.tile([128, 2 * TQ], DT16, tag="et", name="eT")
                    st["ps2"], st["eT"] = ps2, eT
                    for half, ik in ((0, 2 * s), (1, 2 * s + 1)):
                        m = ik - jq * 4
                        q_h = q2[64 * half:64 * half + 64, :]
                        klo = slice(64 * half, 64 * half + 64)
                        if m < 0:  # below diagonal: full tile
                            nc.tensor.matmul(
                                ps2[:, half * TQ:(half + 1) * TQ],
                                k2[klo, ik * 128:(ik + 1) * 128],
                                q_h[:, jq * TQ:(jq + 1) * TQ],
                                start=True, stop=True,
                                tile_position=(64 * half, 0))
                        else:      # diagonal: only columns >= m*128 valid
                            nc.tensor.matmul(
                                ps2[:, half * TQ + m * 128:(half + 1) * TQ],
                                k2[klo, ik * 128:(ik + 1) * 128],
                                q_h[:, jq * TQ + m * 128:(jq + 1) * TQ],
                                start=True, stop=True,
                                tile_position=(64 * half, 0))
                    m_a = 2 * s - jq * 4
                    if m_a < 0:
                        nc.scalar.activation(eT[:], ps2[:], AF.Exp)
                    else:
                        # valid ranges only; trimmed cols are never read
                        a0, b0 = m_a * 128, TQ
                        a1, b1 = TQ + (m_a + 1) * 128, 2 * TQ
                        nc.scalar.activation(eT[:, a0:b0], ps2[:, a0:b0],
                                             AF.Exp)
                        nc.scalar.activation(eT[:, a1:b1], ps2[:, a1:b1],
                                             AF.Exp)
                        # zero the 128x128 triangle at each diagonal block
                        nc.vector.tensor_mul(eT[:, a0:a0 + 128],
                                             eT[:, a0:a0 + 128], tri)
                        nc.vector.tensor_mul(eT[:, a1:a1 + 128],
                                             eT[:, a1:a1 + 128], tri)

                def pv(s, si):
                    eT = st["eT"]
                    po = st["po"]
                    last = (si == len(s_list) - 1)
                    for half, ik in ((0, 2 * s), (1, 2 * s + 1)):
                        m = ik - jq * 4
                        off = m * 128 if m > 0 else 0
                        nc.tensor.matmul(
                            po[:, off:TQ], v_sb[:, ik, :],
                            eT[:, half * TQ + off:(half + 1) * TQ],
                            start=(si == 0 and half == 0),
                            stop=(last and half == 1),
                            skip_group_check=True)

                split_tail = (jq == N_JQ - 1)

                def yout(c0, c1):
                    if "ysb" not in st:
                        st["ysb"] = y_pool.tile([H + 1, TQ], F32, tag="ysb",
                                                name="ysb")
                    y_sb = st["ysb"]
                    nc.vector.tensor_copy(y_sb[:, c0:c1], st["po"][:, c0:c1])
                    nc.sync.dma_start(y_d[:, jq * TQ + c0:jq * TQ + c1],
                                      y_sb[:, c0:c1])

                for si, s in enumerate(s_list):
                    th.append(lambda s=s, si=si: s_exp(s, si))
                    th.append(lambda s=s, si=si: pv(s, si))
                    if split_tail and si == n_pairs - 2:
                        # cols [0:256) take no writes from the last (diagonal)
                        # pair: flush them while it computes
                        th.append(lambda: yout(0, 256))
                if split_tail:
                    th.append(lambda: yout(256, TQ))
                else:
                    th.append(lambda: yout(0, TQ))
                return th

            def weave(a, b, f=1.0):
                out, i, j = [], 0, 0
                na, nb = len(a), len(b)
                while i < na or j < nb:
                    if j >= nb or (i < na and i * nb <= j * na * f):
                        out.append(a[i]); i += 1
                    else:
                        out.append(b[j]); j += 1
                return out

            # ---- schedule -----------------------------------------------
            # w0: qk(0) + v(0)           (wave 0 data only)
            # w1: qk(1)         x attn(0)
            # w2: qk(2) + v(1,2) x attn(1)
            # w3: qk(3) + v(3)   x attn(2)
            # w4: attn(3)
            for t in qk_thunks(0) + v_single_thunks(0):
                t()
            for t in weave(qk_thunks(1), attn_thunks(0)):
                t()
            for t in weave(v_pair_thunks(1, 2) + qk_thunks(2),
                           attn_thunks(1)):
                t()
            for t in weave(v_single_thunks(3) + qk_thunks(3),
                           attn_thunks(2)):
                t()
            for t in attn_thunks(3):
                t()

    nc.compile()
    return nc


def prepare_in_maps(x, wq, bq, wk, bk, wv, bv):
    f16 = np.float16
    f32 = np.float32
    sc = f32(C) ** -0.5
    xT = np.asarray(x).astype(f16).transpose(0, 2, 1)          # [B, C, T]
    # wave-major layout: xw[b, w, p, c*512+t] = xT[b, c*128+p, w*512+t]
    xw = np.ascontiguousarray(
        xT.reshape(B, N_KC, 128, N_JQ, TQ).transpose(0, 3, 2, 1, 4)
        .reshape(B, N_JQ, 128, N_KC * TQ))

    wqk = np.concatenate([np.asarray(wq) * sc, np.asarray(wk)], 1).astype(f16)
    wqk = np.ascontiguousarray(
        wqk.reshape(N_KC, 128, 128).transpose(1, 0, 2)).reshape(128, 1024)
    wv_c = np.asarray(wv).astype(f16)
    wv_c = np.ascontiguousarray(
        wv_c.reshape(N_KC, 128, H).transpose(1, 0, 2)).reshape(128, 512)
    ident = np.concatenate([np.eye(H), np.eye(H)], 0).astype(f16)
    r_i = np.arange(128)[:, None]
    c_i = np.arange(128)[None, :]
    tri = (c_i >= r_i).astype(f16)
    cst = np.ascontiguousarray(
        np.concatenate([wqk, wv_c, ident, tri], 1))
    assert cst.shape == (128, CST_W)

    bqk = np.concatenate([np.asarray(bq) * sc, np.asarray(bk)]) \
        .astype(f32).reshape(128, 1)
    bv2 = np.concatenate([np.asarray(bv), np.asarray(bv)]) \
        .astype(f32).reshape(128, 1)
    shared = {"cst": cst, "bqk": bqk, "bv2": bv2}
    return [{"xw": np.ascontiguousarray(xw[b]), **shared} for b in range(B)]


def postprocess(ys):
    out = np.empty((B, T, H), np.float32)
    for b, y in enumerate(ys):
        out[b] = (y[:H] / y[H:H + 1]).T
    return out


def kernel(**inputs):
    global _CACHED_NC
    if _CACHED_NC is None:
        _CACHED_NC = build_program(reps=1)
    nc = _CACHED_NC
    in_maps = prepare_in_maps(
        inputs["x"], inputs["wq"], inputs["bq"], inputs["wk"],
        inputs["bk"], inputs["wv"], inputs["bv"])
    res = run_bass_kernel_spmd(nc, in_maps, core_ids=list(range(N_CORES)))
    return postprocess([r["y"] for r in res.results])


if __name__ == "__main__":
    rng = np.random.default_rng(0)
    demo = {
        "x": rng.standard_normal((B, T, C), dtype=np.float32),
        "wq": rng.standard_normal((C, H), dtype=np.float32) * 0.02,
        "bq": rng.standard_normal((H,), dtype=np.float32) * 0.02,
        "wk": rng.standard_normal((C, H), dtype=np.float32) * 0.02,
        "bk": rng.standard_normal((H,), dtype=np.float32) * 0.02,
        "wv": rng.standard_normal((C, H), dtype=np.float32) * 0.02,
        "bv": rng.standard_normal((H,), dtype=np.float32) * 0.02,
    }
    out = kernel(**demo)
    print("kernel output:", out.shape, out.dtype)


# revision 20
# speedup vs baseline: 1.1521x; 1.1521x over previous
"""Single-head causal attention on 8 Trainium2 NeuronCores.

Problem: x[8, 2048, 1024] f32, per-head projections (H=64), causal
softmax attention scaled by C**-0.5.

Strategy: data-parallel over batch (1 batch element per core). Per core
everything is kept in transposed layout so no fp32 on-chip transposes of
large tensors are needed:

  - host pre-casts x to fp16 and lays it out wave-major as
    xw[w][p][c*512+t] so each of the 4 waves is one DMA instruction with
    8KB-contiguous reads per partition (max HBM efficiency); waves are
    issued on 4 different engine queues so issue cost is not serialized
  - Q^T/K^T [64, T] computed with the weight matrices stationary; Q and
    K packed into one PSUM tile via PE column tiling (concurrent on HW)
  - V^T computed the same way; tj pairs column-tiled where both waves
    are resident; V then PE-transposed to [Tk, H] for the PV matmuls
  - S^T tiles [128 Tk, 512 Tq] = (K^T chunk).T @ Q^T, two at a time via
    PE row tiling into one two-bank PSUM tile
  - causal masking without any mask loads: diagonal S blocks are
    column-trimmed in the matmul itself, exp is emitted only over the
    valid column ranges, and the 128x128 triangle at the diagonal is
    zeroed by one small DVE multiply with an SBUF triangle constant
  - softmax runs without max-subtraction (logits are O(1) by
    construction)
  - P@V: V tiles [128 Tk, 65] carry an appended ones column, so the
    softmax denominator falls out of the same PSUM accumulation as the
    numerator; PV accumulation runs diagonal-first so trimmed moving
    ranges never conflict with the accumulation start flag
  - a short chain of dummy matmuls warms the PE clock (HAM ramp) during
    the DMA-bound startup window
  - normalization (divide by denominator row) + final transpose on host

Outputs are returned as float32 [8, 2048, 64].
"""

import numpy as np

import concourse.mybir as mybir
import concourse.tile as tile
from concourse import bacc
from concourse.bass_utils import run_bass_kernel_spmd

B, T, C, H = 8, 2048, 1024, 64
N_CORES = 8
TQ = 512          # Tq chunk (one fp32 PSUM bank)
N_JQ = T // TQ    # 4
N_TK = T // 128   # 16
N_KC = C // 128   # 8  contraction chunks for projections

DT16 = mybir.dt.float16  # fp16: same PE/DVE/DMA speed as bf16, 8 more mantissa bits
F32 = mybir.dt.float32
AF = mybir.ActivationFunctionType

WARM_N = 14       # PE clock pre-warm matmuls during the DMA-bound startup

# column offsets inside the packed fp16 constant tensor
CST_WQK = 0            # [128, 8*128] striped [wq | wk] per c-chunk
CST_WV = 1024          # [128, 8*64] striped wv per c-chunk
CST_ID = 1536          # [128, 64] eye(64) stacked twice
CST_TRI = 1600         # [128, 128] upper triangle (col >= row) 0/1
CST_W = 1728

_CACHED_NC = None


def build_program(reps=1):
    nc = bacc.Bacc("TRN2", target_bir_lowering=False, debug=False,
                   num_devices=N_CORES)

    xw_d = nc.dram_tensor("xw", [N_JQ, 128, N_KC * TQ], DT16,
                          kind="ExternalInput").ap()
    cst_d = nc.dram_tensor("cst", [128, CST_W], DT16,
                           kind="ExternalInput").ap()
    bqk_d = nc.dram_tensor("bqk", [128, 1], F32, kind="ExternalInput").ap()
    bv2_d = nc.dram_tensor("bv2", [128, 1], F32, kind="ExternalInput").ap()
    y_d = nc.dram_tensor("y", [H + 1, T], F32, kind="ExternalOutput").ap()

    with tile.TileContext(nc) as tc:
        with (
            tc.tile_pool(name="sb", bufs=1) as sb,
            tc.tile_pool(name="et", bufs=6) as et_pool,
            tc.tile_pool(name="ysb", bufs=2) as y_pool,
            tc.tile_pool(name="ps_proj", bufs=2, space="PSUM") as ps_proj,
            tc.tile_pool(name="ps_s", bufs=2, space="PSUM") as ps_s,
            tc.tile_pool(name="ps_o", bufs=2, space="PSUM") as ps_o,
        ):
            cst = sb.tile([128, CST_W], DT16, tag="cst")
            bqk_sb = sb.tile([128, 1], F32, tag="bqk")
            bv2_sb = sb.tile([128, 1], F32, tag="bv2")
            xw = [sb.tile([128, N_KC * TQ], DT16, tag=f"xw{w}",
                          name=f"xw{w}")
                  for w in range(N_JQ)]
            q1 = sb.tile([64, T], DT16, tag="q1")
            k1 = sb.tile([64, T], DT16, tag="k1")
            vT = sb.tile([128, T // 2], DT16, tag="vT")
            v_sb = sb.tile([128, N_TK, H + 1], DT16, tag="v")
            warm = sb.tile([128, TQ], DT16, tag="warm")

            def wqk_ap(c, lo, hi):
                return cst[:, CST_WQK + c * 128 + lo:CST_WQK + c * 128 + hi]

            def wv_ap(c):
                return cst[:, CST_WV + c * H:CST_WV + (c + 1) * H]

            def ident_ap(half):
                return cst[64 * half:64 * half + 64, CST_ID:CST_ID + H]

            tri = cst[:, CST_TRI:CST_TRI + 128]

            # ---- DMA issue ----------------------------------------------
            # DMAs on one engine queue serialize on one hardware ring, and
            # ring rate scales with contiguous element size (~45GB/s at
            # 2KB, ~2x at 8KB).  Waves are therefore split by PARTITION
            # half — each piece keeps the full 8KB/partition contiguous
            # run of the wave-major layout — and spread so each queue's
            # FIFO delivers in consumption order.  Tiny bias loads go
            # first so they never sit behind megabytes of x.
            def xq(w, q):
                sl = slice(q * 2 * TQ, (q + 1) * 2 * TQ)
                return xw[w][:, sl], xw_d[w][:, sl]

            def xh(w, h):
                sl = slice(h * 4 * TQ, (h + 1) * 4 * TQ)
                return xw[w][:, sl], xw_d[w][:, sl]

            nc.sync.dma_start(*xq(0, 0))
            nc.gpsimd.dma_start(bqk_sb[:], bqk_d)
            nc.scalar.dma_start(cst[:, 0:CST_WV], cst_d[:, 0:CST_WV])
            nc.vector.memset(warm[:], 0.5)
            nc.vector.memset(v_sb[:], 1.0)
            nc.gpsimd.dma_start(bv2_sb[:], bv2_d)
            nc.gpsimd.dma_start(cst[:, CST_WV:CST_W], cst_d[:, CST_WV:CST_W])
            nc.scalar.dma_start(*xq(0, 1))
            nc.gpsimd.dma_start(*xq(0, 2))
            nc.sync.dma_start(*xq(1, 0))
            nc.gpsimd.dma_start(*xq(0, 3))
            nc.sync.dma_start(*xq(1, 2))
            nc.scalar.dma_start(*xh(2, 0))
            nc.gpsimd.dma_start(*xq(1, 1))
            nc.sync.dma_start(*xh(2, 1))
            nc.scalar.dma_start(*xh(3, 0))
            nc.gpsimd.dma_start(*xq(1, 3))
            nc.sync.dma_start(*xh(3, 1))

            # ---- PE clock warm-up while DMA streams in ------------------
            if WARM_N:
                wps = ps_proj.tile([64, TQ], F32, tag="proj", name="warm_ps")
                for _w in range(WARM_N):
                    nc.tensor.matmul(wps[:], warm[:, 0:64], warm[:],
                                     start=True, stop=True,
                                     tile_position=(0, 0),
                                     skip_group_check=True)

            # ---- projection thunks --------------------------------------
            def qk_thunks(tj):
                sl = slice(tj * TQ, (tj + 1) * TQ)
                st = {}
                th = []

                def qk_mm(c):
                    if c == 0:
                        st["ps"] = ps_proj.tile([128, TQ], F32, tag="proj",
                                                name="ps_qk")
                    ps = st["ps"]
                    nc.tensor.matmul(ps[0:64, :], wqk_ap(c, 0, 64),
                                     xw[tj][:, c * TQ:(c + 1) * TQ],
                                     start=(c == 0), stop=(c == N_KC - 1),
                                     tile_position=(0, 0),
                                     skip_group_check=True)
                    nc.tensor.matmul(ps[64:128, :], wqk_ap(c, 64, 128),
                                     xw[tj][:, c * TQ:(c + 1) * TQ],
                                     start=(c == 0), stop=(c == N_KC - 1),
                                     tile_position=(0, 64),
                                     skip_group_check=True)
                for c in range(N_KC):
                    th.append(lambda c=c: qk_mm(c))

                def qk_epi():
                    ps = st["ps"]
                    nc.vector.tensor_scalar_add(q1[:, sl], ps[0:64, :],
                                                bqk_sb[0:64, :])
                    nc.vector.tensor_scalar_add(k1[:, sl], ps[64:128, :],
                                                bqk_sb[64:128, :])
                th.append(qk_epi)
                return th

            def v_tr_thunk(tj, st):
                # transpose 4 [64,128] V^T blocks of chunk tj into v_sb
                def tr():
                    half = tj % 2
                    base = (tj // 2) * TQ
                    pt = ps_proj.tile([128, 4, H], DT16, tag="proj",
                                      name=f"pt{tj}")
                    for r in range(4):
                        nc.tensor.transpose(
                            pt[:, r, :],
                            vT[64 * half:64 * half + 64,
                               base + r * 128:base + (r + 1) * 128],
                            ident_ap(half))
                    nc.vector.tensor_copy(v_sb[:, 4 * tj:4 * tj + 4, 0:H],
                                          pt[:, :, :])
                return tr

            def v_single_thunks(tj):
                st = {}
                th = []
                half = tj % 2

                def v_mm(c):
                    if c == 0:
                        st["psv"] = ps_proj.tile([128, TQ], F32, tag="proj",
                                                 name=f"ps_v{tj}")
                    nc.tensor.matmul(st["psv"][64 * half:64 * half + 64, :],
                                     wv_ap(c), xw[tj][:, c * TQ:(c + 1) * TQ],
                                     start=(c == 0), stop=(c == N_KC - 1),
                                     tile_position=(0, 64 * half),
                                     skip_group_check=True)
                for c in range(N_KC):
                    th.append(lambda c=c: v_mm(c))

                def v_epi():
                    nc.vector.tensor_scalar_add(
                        vT[64 * half:64 * half + 64,
                           (tj // 2) * TQ:(tj // 2 + 1) * TQ],
                        st["psv"][64 * half:64 * half + 64, :],
                        bv2_sb[64 * half:64 * half + 64, :])
                th.append(v_epi)
                th.append(v_tr_thunk(tj, st))
                return th

            def v_pair_thunks(tja, tjb):
                # tja even, tjb = tja+1: column-tiled pair, both waves loaded
                st = {}
                th = []

                def v_mm(c):
                    if c == 0:
                        st["psv"] = ps_proj.tile([128, TQ], F32, tag="proj",
                                                 name=f"ps_v{tja}{tjb}")
                    for tj in (tja, tjb):
                        half = tj % 2
                        nc.tensor.matmul(
                            st["psv"][64 * half:64 * half + 64, :], wv_ap(c),
                            xw[tj][:, c * TQ:(c + 1) * TQ],
                            start=(c == 0), stop=(c == N_KC - 1),
                            tile_position=(0, 64 * half),
                            skip_group_check=True)
                for c in range(N_KC):
                    th.append(lambda c=c: v_mm(c))

                def v_epi():
                    for tj in (tja, tjb):
                        half = tj % 2
                        nc.vector.tensor_scalar_add(
                            vT[64 * half:64 * half + 64,
                               (tj // 2) * TQ:(tj // 2 + 1) * TQ],
                            st["psv"][64 * half:64 * half + 64, :],
                            bv2_sb[64 * half:64 * half + 64, :])
                th.append(v_epi)
                th.append(v_tr_thunk(tja, st))
                th.append(v_tr_thunk(tjb, st))
                return th

            # ---- attention thunks ---------------------------------------
            def attn_thunks(jq):
                n_ik = (jq + 1) * 4
                n_pairs = n_ik // 2
                # natural order: the accumulation start flag lands on the
                # full-range s=0 PV; the trimmed diagonal pairs come last
                # (max slack for the freshest V transposes, short tail)
                s_list = list(range(n_pairs))
                st = {}
                th = []

                def s_exp(s, si):
                    if si == 0:
                        st["po"] = ps_o.tile([H + 1, TQ], F32, tag="out",
                                             name="po")
                    ps2 = ps_s.tile([128, 2 * TQ], F32, tag="s")
                    eT = et_pool.tile([128, 2 * TQ], DT16, tag="et", name="eT")
                    st["ps2"], st["eT"] = ps2, eT
                    for half, ik in ((0, 2 * s), (1, 2 * s + 1)):
                        m = ik - jq * 4
                        # two column tiles per k-chunk share the same moving
                        # q stream: 2 rows/cycle with no q/k duplication
                        off = m * 128 if m >= 0 else 0
                        rhs = q1[:, jq * TQ + off:(jq + 1) * TQ]
                        for ct in (0, 1):
                            nc.tensor.matmul(
                                ps2[64 * ct:64 * ct + 64,
                                    half * TQ + off:(half + 1) * TQ],
                                k1[:, ik * 128 + 64 * ct:
                                   ik * 128 + 64 * ct + 64],
                                rhs,
                                start=True, stop=True,
                                tile_position=(0, 64 * ct))
                    m_a = 2 * s - jq * 4
                    if m_a < 0:
                        nc.scalar.activation(eT[:], ps2[:], AF.Exp)
                    else:
                        # valid ranges only; trimmed cols are never read
                        a0, b0 = m_a * 128, TQ
                        a1, b1 = TQ + (m_a + 1) * 128, 2 * TQ
                        nc.scalar.activation(eT[:, a0:b0], ps2[:, a0:b0],
                                             AF.Exp)
                        nc.scalar.activation(eT[:, a1:b1], ps2[:, a1:b1],
                                             AF.Exp)
                        # zero the 128x128 triangle at each diagonal block
                        nc.vector.tensor_mul(eT[:, a0:a0 + 128],
                                             eT[:, a0:a0 + 128], tri)
                        nc.vector.tensor_mul(eT[:, a1:a1 + 128],
                                             eT[:, a1:a1 + 128], tri)

                def pv(s, si):
                    eT = st["eT"]
                    po = st["po"]
                    last = (si == len(s_list) - 1)
                    for half, ik in ((0, 2 * s), (1, 2 * s + 1)):
                        m = ik - jq * 4
                        off = m * 128 if m > 0 else 0
                        nc.tensor.matmul(
                            po[:, off:TQ], v_sb[:, ik, :],
                            eT[:, half * TQ + off:(half + 1) * TQ],
                            start=(si == 0 and half == 0),
                            stop=(last and half == 1),
                            skip_group_check=True)

                split_tail = (jq == N_JQ - 1)

                def yout(c0, c1):
                    if "ysb" not in st:
                        st["ysb"] = y_pool.tile([H + 1, TQ], F32, tag="ysb",
                                                name="ysb")
                    y_sb = st["ysb"]
                    nc.vector.tensor_copy(y_sb[:, c0:c1], st["po"][:, c0:c1])
                    nc.sync.dma_start(y_d[:, jq * TQ + c0:jq * TQ + c1],
                                      y_sb[:, c0:c1])

                for si, s in enumerate(s_list):
                    th.append(lambda s=s, si=si: s_exp(s, si))
                    th.append(lambda s=s, si=si: pv(s, si))
                    if split_tail and si == n_pairs - 2:
                        # cols [0:256) take no writes from the last (diagonal)
                        # pair: flush them while it computes
                        th.append(lambda: yout(0, 256))
                if split_tail:
                    th.append(lambda: yout(256, TQ))
                else:
                    th.append(lambda: yout(0, TQ))
                return th

            def weave(a, b, f=1.0):
                out, i, j = [], 0, 0
                na, nb = len(a), len(b)
                while i < na or j < nb:
                    if j >= nb or (i < na and i * nb <= j * na * f):
                        out.append(a[i]); i += 1
                    else:
                        out.append(b[j]); j += 1
                return out

            # ---- schedule -----------------------------------------------
            # w0: qk(0) + v(0)           (wave 0 data only)
            # w1: qk(1)         x attn(0)
            # w2: qk(2) + v(1,2) x attn(1)
            # w3: qk(3) + v(3)   x attn(2)
            # w4: attn(3)
            for t in qk_thunks(0) + v_single_thunks(0):
                t()
            for t in weave(qk_thunks(1), attn_thunks(0)):
                t()
            for t in weave(v_pair_thunks(1, 2) + qk_thunks(2),
                           attn_thunks(1)):
                t()
            for t in weave(v_single_thunks(3) + qk_thunks(3),
                           attn_thunks(2)):
                t()
            for t in attn_thunks(3):
                t()

    nc.compile()
    return nc


def prepare_in_maps(x, wq, bq, wk, bk, wv, bv):
    f16 = np.float16
    f32 = np.float32
    sc = f32(C) ** -0.5
    xT = np.asarray(x).astype(f16).transpose(0, 2, 1)          # [B, C, T]
    # wave-major layout: xw[b, w, p, c*512+t] = xT[b, c*128+p, w*512+t]
    xw = np.ascontiguousarray(
        xT.reshape(B, N_KC, 128, N_JQ, TQ).transpose(0, 3, 2, 1, 4)
        .reshape(B, N_JQ, 128, N_KC * TQ))

    wqk = np.concatenate([np.asarray(wq) * sc, np.asarray(wk)], 1).astype(f16)
    wqk = np.ascontiguousarray(
        wqk.reshape(N_KC, 128, 128).transpose(1, 0, 2)).reshape(128, 1024)
    wv_c = np.asarray(wv).astype(f16)
    wv_c = np.ascontiguousarray(
        wv_c.reshape(N_KC, 128, H).transpose(1, 0, 2)).reshape(128, 512)
    ident = np.concatenate([np.eye(H), np.eye(H)], 0).astype(f16)
    r_i = np.arange(128)[:, None]
    c_i = np.arange(128)[None, :]
    tri = (c_i >= r_i).astype(f16)
    cst = np.ascontiguousarray(
        np.concatenate([wqk, wv_c, ident, tri], 1))
    assert cst.shape == (128, CST_W)

    bqk = np.concatenate([np.asarray(bq) * sc, np.asarray(bk)]) \
        .astype(f32).reshape(128, 1)
    bv2 = np.concatenate([np.asarray(bv), np.asarray(bv)]) \
        .astype(f32).reshape(128, 1)
    shared = {"cst": cst, "bqk": bqk, "bv2": bv2}
    return [{"xw": np.ascontiguousarray(xw[b]), **shared} for b in range(B)]


def postprocess(ys):
    out = np.empty((B, T, H), np.float32)
    for b, y in enumerate(ys):
        out[b] = (y[:H] / y[H:H + 1]).T
    return out


def kernel(**inputs):
    global _CACHED_NC
    if _CACHED_NC is None:
        _CACHED_NC = build_program(reps=1)
    nc = _CACHED_NC
    in_maps = prepare_in_maps(
        inputs["x"], inputs["wq"], inputs["bq"], inputs["wk"],
        inputs["bk"], inputs["wv"], inputs["bv"])
    res = run_bass_kernel_spmd(nc, in_maps, core_ids=list(range(N_CORES)))
    return postprocess([r["y"] for r in res.results])


if __name__ == "__main__":
    rng = np.random.default_rng(0)
    demo = {
        "x": rng.standard_normal((B, T, C), dtype=np.float32),
        "wq": rng.standard_normal((C, H), dtype=np.float32) * 0.02,
        "bq": rng.standard_normal((H,), dtype=np.float32) * 0.02,
        "wk": rng.standard_normal((C, H), dtype=np.float32) * 0.02,
        "bk": rng.standard_normal((H,), dtype=np.float32) * 0.02,
        "wv": rng.standard_normal((C, H), dtype=np.float32) * 0.02,
        "bv": rng.standard_normal((H,), dtype=np.float32) * 0.02,
    }
    out = kernel(**demo)
    print("kernel output:", out.shape, out.dtype)


# revision 27
# speedup vs baseline: 1.3170x; 1.1431x over previous
"""Single-head causal attention on 8 Trainium2 NeuronCores.

Problem: x[8, 2048, 1024] f32, per-head projections (H=64), causal
softmax attention scaled by C**-0.5.

Strategy: data-parallel over batch (1 batch element per core). Per core
everything is kept in transposed layout so no fp32 on-chip transposes of
large tensors are needed:

  - host pre-casts x to fp16 and lays it out wave-major as
    xw[w][p][c*512+t] so each of the 4 waves is one DMA instruction with
    8KB-contiguous reads per partition (max HBM efficiency); waves are
    issued on 4 different engine queues so issue cost is not serialized
  - Q^T/K^T [64, T] computed with the weight matrices stationary; Q and
    K packed into one PSUM tile via PE column tiling (concurrent on HW)
  - V^T computed the same way; tj pairs column-tiled where both waves
    are resident; V then PE-transposed to [Tk, H] for the PV matmuls
  - S^T tiles [128 Tk, 512 Tq] = (K^T chunk).T @ Q^T, two at a time via
    PE row tiling into one two-bank PSUM tile
  - causal masking without any mask loads: diagonal S blocks are
    column-trimmed in the matmul itself, exp is emitted only over the
    valid column ranges, and the 128x128 triangle at the diagonal is
    zeroed by one small DVE multiply with an SBUF triangle constant
  - softmax runs without max-subtraction (logits are O(1) by
    construction)
  - P@V: V tiles [128 Tk, 65] carry an appended ones column, so the
    softmax denominator falls out of the same PSUM accumulation as the
    numerator; PV accumulation runs diagonal-first so trimmed moving
    ranges never conflict with the accumulation start flag
  - a short chain of dummy matmuls warms the PE clock (HAM ramp) during
    the DMA-bound startup window
  - normalization (divide by denominator row) + final transpose on host

Outputs are returned as float32 [8, 2048, 64].
"""

import numpy as np

import concourse.mybir as mybir
import concourse.tile as tile
from concourse import bacc
from concourse.bass_utils import run_bass_kernel_spmd

B, T, C, H = 8, 2048, 1024, 64
N_CORES = 8
TQ = 512          # Tq chunk (one fp32 PSUM bank)
N_JQ = T // TQ    # 4
N_TK = T // 128   # 16
N_KC = C // 128   # 8  contraction chunks for projections

DT16 = mybir.dt.float16  # fp16: same PE/DVE/DMA speed as bf16, 8 more mantissa bits
F32 = mybir.dt.float32
AF = mybir.ActivationFunctionType

WARM_N = 6       # PE clock pre-warm matmuls during the DMA-bound startup

# column offsets inside the packed fp16 constant tensor
CST_WQK = 0            # [128, 8*128] striped [wq | wk] per c-chunk
CST_WV = 1024          # [128, 8*64] striped wv per c-chunk
CST_ID = 1536          # [128, 64] eye(64) stacked twice
CST_TRI = 1600         # [128, 128] upper triangle (col >= row) 0/1
CST_W = 1728

_CACHED_NC = None


def build_program(reps=1):
    nc = bacc.Bacc("TRN2", target_bir_lowering=False, debug=False,
                   num_devices=N_CORES)

    xw_d = nc.dram_tensor("xw", [N_JQ, 128, N_KC * TQ], DT16,
                          kind="ExternalInput").ap()
    cst_d = nc.dram_tensor("cst", [128, CST_W], DT16,
                           kind="ExternalInput").ap()
    bqk_d = nc.dram_tensor("bqk", [128, 1], F32, kind="ExternalInput").ap()
    bv2_d = nc.dram_tensor("bv2", [128, 1], F32, kind="ExternalInput").ap()
    y_d = nc.dram_tensor("y", [H + 1, T], DT16, kind="ExternalOutput").ap()

    with tile.TileContext(nc) as tc:
        with (
            tc.tile_pool(name="sb", bufs=1) as sb,
            tc.tile_pool(name="et", bufs=6) as et_pool,
            tc.tile_pool(name="ysb", bufs=2) as y_pool,
            tc.tile_pool(name="ps_proj", bufs=2, space="PSUM") as ps_proj,
            tc.tile_pool(name="ps_s", bufs=2, space="PSUM") as ps_s,
            tc.tile_pool(name="ps_o", bufs=2, space="PSUM") as ps_o,
        ):
            cst = sb.tile([128, CST_W], DT16, tag="cst")
            bqk_sb = sb.tile([128, 1], F32, tag="bqk")
            bv2_sb = sb.tile([128, 1], F32, tag="bv2")
            xw = [sb.tile([128, N_KC * TQ], DT16, tag=f"xw{w}",
                          name=f"xw{w}")
                  for w in range(N_JQ)]
            q2 = sb.tile([128, T], DT16, tag="q2")
            k2 = sb.tile([128, T], DT16, tag="k2")
            vT = sb.tile([128, T // 2], DT16, tag="vT")
            v_sb = sb.tile([128, N_TK, H + 1], DT16, tag="v")
            warm = sb.tile([128, TQ], DT16, tag="warm")

            def wqk_ap(c, lo, hi):
                return cst[:, CST_WQK + c * 128 + lo:CST_WQK + c * 128 + hi]

            def wv_ap(c):
                return cst[:, CST_WV + c * H:CST_WV + (c + 1) * H]

            def ident_ap(half):
                return cst[64 * half:64 * half + 64, CST_ID:CST_ID + H]

            tri = cst[:, CST_TRI:CST_TRI + 128]

            # ---- DMA issue ----------------------------------------------
            # DMAs on one engine queue serialize on one hardware ring, and
            # ring rate scales with contiguous element size (~45GB/s at
            # 2KB, ~2x at 8KB).  Waves are therefore split by PARTITION
            # half — each piece keeps the full 8KB/partition contiguous
            # run of the wave-major layout — and spread so each queue's
            # FIFO delivers in consumption order.  Tiny bias loads go
            # first so they never sit behind megabytes of x.
            def xq(w, q):
                sl = slice(q * 2 * TQ, (q + 1) * 2 * TQ)
                return xw[w][:, sl], xw_d[w][:, sl]

            def xh(w, h):
                sl = slice(h * 4 * TQ, (h + 1) * 4 * TQ)
                return xw[w][:, sl], xw_d[w][:, sl]

            def xc(w, c):
                sl = slice(c * TQ, (c + 1) * TQ)
                return xw[w][:, sl], xw_d[w][:, sl]

            # wave 0 goes as single chunks zipped with per-chunk QK work;
            # wqk is split so chunk 0's weights land first
            nc.sync.dma_start(*xc(0, 0))
            nc.gpsimd.dma_start(bqk_sb[:], bqk_d)
            nc.scalar.dma_start(cst[:, 0:512], cst_d[:, 0:512])
            nc.vector.memset(warm[:], 0.5)
            nc.vector.memset(v_sb[:], 1.0)
            nc.gpsimd.dma_start(bv2_sb[:], bv2_d)
            nc.scalar.dma_start(cst[:, 512:CST_WV], cst_d[:, 512:CST_WV])
            nc.gpsimd.dma_start(cst[:, CST_WV:CST_W], cst_d[:, CST_WV:CST_W])
            nc.sync.dma_start(*xc(0, 2))
            nc.scalar.dma_start(*xc(0, 4))
            nc.gpsimd.dma_start(*xc(0, 1))
            nc.sync.dma_start(*xc(0, 5))
            nc.scalar.dma_start(*xc(0, 6))
            nc.gpsimd.dma_start(*xc(0, 3))
            nc.sync.dma_start(*xq(1, 0))
            nc.gpsimd.dma_start(*xc(0, 7))
            nc.sync.dma_start(*xq(1, 2))
            nc.scalar.dma_start(*xh(2, 0))
            nc.gpsimd.dma_start(*xq(1, 1))
            nc.sync.dma_start(*xh(2, 1))
            nc.scalar.dma_start(*xh(3, 0))
            nc.gpsimd.dma_start(*xq(1, 3))
            nc.gpsimd.dma_start(*xh(3, 1))

            # ---- PE clock warm-up while DMA streams in ------------------
            if WARM_N:
                wps = ps_proj.tile([64, TQ], F32, tag="proj", name="warm_ps")
                for _w in range(WARM_N):
                    nc.tensor.matmul(wps[:], warm[:, 0:64], warm[:],
                                     start=True, stop=True,
                                     tile_position=(0, 0),
                                     skip_group_check=True)

            # ---- projection thunks --------------------------------------
            # wave-0 chunks are consumed in DMA arrival order
            ORDER0 = [0, 4, 2, 1, 6, 5, 3, 7]

            def qk_thunks(tj):
                sl = slice(tj * TQ, (tj + 1) * TQ)
                order = ORDER0 if tj == 0 else list(range(N_KC))
                st = {}
                th = []

                def qk_mm(c, first, last):
                    if first:
                        st["ps"] = ps_proj.tile([128, TQ], F32, tag="proj",
                                                name="ps_qk")
                    ps = st["ps"]
                    nc.tensor.matmul(ps[0:64, :], wqk_ap(c, 0, 64),
                                     xw[tj][:, c * TQ:(c + 1) * TQ],
                                     start=first, stop=last,
                                     tile_position=(0, 0),
                                     skip_group_check=True)
                    nc.tensor.matmul(ps[64:128, :], wqk_ap(c, 64, 128),
                                     xw[tj][:, c * TQ:(c + 1) * TQ],
                                     start=first, stop=last,
                                     tile_position=(0, 64),
                                     skip_group_check=True)
                for i, c in enumerate(order):
                    th.append(lambda c=c, i=i: qk_mm(c, i == 0,
                                                     i == N_KC - 1))

                def qk_epi():
                    ps = st["ps"]
                    nc.vector.tensor_scalar_add(q2[0:64, sl], ps[0:64, :],
                                                bqk_sb[0:64, :])
                    nc.vector.tensor_scalar_add(k2[64:128, sl], ps[64:128, :],
                                                bqk_sb[64:128, :])
                    nc.vector.tensor_copy(q2[64:128, sl], q2[0:64, sl])
                    nc.vector.tensor_copy(k2[0:64, sl], k2[64:128, sl])
                th.append(qk_epi)
                return th

            def v_tr_thunk(tj, st):
                # transpose 4 [64,128] V^T blocks of chunk tj into v_sb
                def tr():
                    half = tj % 2
                    base = (tj // 2) * TQ
                    pt = ps_proj.tile([128, 4, H], DT16, tag="proj",
                                      name=f"pt{tj}")
                    for r in range(4):
                        nc.tensor.transpose(
                            pt[:, r, :],
                            vT[64 * half:64 * half + 64,
                               base + r * 128:base + (r + 1) * 128],
                            ident_ap(half))
                    nc.vector.tensor_copy(v_sb[:, 4 * tj:4 * tj + 4, 0:H],
                                          pt[:, :, :])
                return tr

            def v_single_thunks(tj):
                st = {}
                th = []
                half = tj % 2
                order = ORDER0 if tj == 0 else list(range(N_KC))

                def v_mm(c, first, last):
                    if first:
                        st["psv"] = ps_proj.tile([128, TQ], F32, tag="proj",
                                                 name=f"ps_v{tj}")
                    nc.tensor.matmul(st["psv"][64 * half:64 * half + 64, :],
                                     wv_ap(c), xw[tj][:, c * TQ:(c + 1) * TQ],
                                     start=first, stop=last,
                                     tile_position=(0, 64 * half),
                                     skip_group_check=True)
                for i, c in enumerate(order):
                    th.append(lambda c=c, i=i: v_mm(c, i == 0,
                                                    i == N_KC - 1))

                def v_epi():
                    nc.vector.tensor_scalar_add(
                        vT[64 * half:64 * half + 64,
                           (tj // 2) * TQ:(tj // 2 + 1) * TQ],
                        st["psv"][64 * half:64 * half + 64, :],
                        bv2_sb[64 * half:64 * half + 64, :])
                th.append(v_epi)
                th.append(v_tr_thunk(tj, st))
                return th

            def v_pair_thunks(tja, tjb):
                # tja even, tjb = tja+1: column-tiled pair, both waves loaded
                st = {}
                th = []

                def v_mm(c):
                    if c == 0:
                        st["psv"] = ps_proj.tile([128, TQ], F32, tag="proj",
                                                 name=f"ps_v{tja}{tjb}")
                    for tj in (tja, tjb):
                        half = tj % 2
                        nc.tensor.matmul(
                            st["psv"][64 * half:64 * half + 64, :], wv_ap(c),
                            xw[tj][:, c * TQ:(c + 1) * TQ],
                            start=(c == 0), stop=(c == N_KC - 1),
                            tile_position=(0, 64 * half),
                            skip_group_check=True)
                for c in range(N_KC):
                    th.append(lambda c=c: v_mm(c))

                def v_epi():
                    for tj in (tja, tjb):
                        half = tj % 2
                        nc.vector.tensor_scalar_add(
                            vT[64 * half:64 * half + 64,
                               (tj // 2) * TQ:(tj // 2 + 1) * TQ],
                            st["psv"][64 * half:64 * half + 64, :],
                            bv2_sb[64 * half:64 * half + 64, :])
                th.append(v_epi)
                th.append(v_tr_thunk(tja, st))
                th.append(v_tr_thunk(tjb, st))
                return th

            # ---- attention thunks ---------------------------------------
            def attn_thunks(jq):
                n_ik = (jq + 1) * 4
                n_pairs = n_ik // 2
                # natural order: the accumulation start flag lands on the
                # full-range s=0 PV; the trimmed diagonal pairs come last
                # (max slack for the freshest V transposes, short tail)
                s_list = list(range(n_pairs))
                st = {}
                th = []

                def s_exp(s, si):
                    if si == 0:
                        st["po"] = ps_o.tile([H + 1, TQ], F32, tag="out",
                                             name="po")
                    ps2 = ps_s.tile([128, 2 * TQ], F32, tag="s")
                    eT = et_pool.tile([128, 2 * TQ], DT16, tag="et", name="eT")
                    st["ps2"], st["eT"] = ps2, eT
                    for half, ik in ((0, 2 * s), (1, 2 * s + 1)):
                        m = ik - jq * 4
                        off = m * 128 if m >= 0 else 0
                        q_h = q2[64 * half:64 * half + 64, :]
                        klo = slice(64 * half, 64 * half + 64)
                        nc.tensor.matmul(
                            ps2[:, half * TQ + off:(half + 1) * TQ],
                            k2[klo, ik * 128:(ik + 1) * 128],
                            q_h[:, jq * TQ + off:(jq + 1) * TQ],
                            start=True, stop=True,
                            tile_position=(64 * half, 0))
                    m_a = 2 * s - jq * 4
                    if m_a < 0:
                        nc.scalar.activation(eT[:], ps2[:], AF.Exp)
                    else:
                        # valid ranges only; trimmed cols are never read
                        a0, b0 = m_a * 128, TQ
                        a1, b1 = TQ + (m_a + 1) * 128, 2 * TQ
                        nc.scalar.activation(eT[:, a0:b0], ps2[:, a0:b0],
                                             AF.Exp)
                        nc.scalar.activation(eT[:, a1:b1], ps2[:, a1:b1],
                                             AF.Exp)
                        # zero the 128x128 triangle at each diagonal block
                        nc.vector.tensor_mul(eT[:, a0:a0 + 128],
                                             eT[:, a0:a0 + 128], tri)
                        nc.vector.tensor_mul(eT[:, a1:a1 + 128],
                                             eT[:, a1:a1 + 128], tri)

                def pv(s, si):
                    eT = st["eT"]
                    po = st["po"]
                    last = (si == len(s_list) - 1)
                    for half, ik in ((0, 2 * s), (1, 2 * s + 1)):
                        m = ik - jq * 4
                        off = m * 128 if m > 0 else 0
                        nc.tensor.matmul(
                            po[:, off:TQ], v_sb[:, ik, :],
                            eT[:, half * TQ + off:(half + 1) * TQ],
                            start=(si == 0 and half == 0),
                            stop=(last and half == 1),
                            skip_group_check=True)

                split_tail = (jq == N_JQ - 1)

                def yout(c0, c1):
                    if "ysb" not in st:
                        st["ysb"] = y_pool.tile([H + 1, TQ], DT16, tag="ysb",
                                                name="ysb")
                    y_sb = st["ysb"]
                    nc.vector.tensor_copy(y_sb[:, c0:c1], st["po"][:, c0:c1])
                    nc.sync.dma_start(y_d[:, jq * TQ + c0:jq * TQ + c1],
                                      y_sb[:, c0:c1])

                for si, s in enumerate(s_list):
                    th.append(lambda s=s, si=si: s_exp(s, si))
                    th.append(lambda s=s, si=si: pv(s, si))
                    if split_tail and si == n_pairs - 2:
                        # cols [0:256) take no writes from the last (diagonal)
                        # pair: flush them while it computes
                        th.append(lambda: yout(0, 256))
                if split_tail:
                    th.append(lambda: yout(256, TQ))
                else:
                    th.append(lambda: yout(0, TQ))
                return th

            def weave(a, b, f=1.0):
                out, i, j = [], 0, 0
                na, nb = len(a), len(b)
                while i < na or j < nb:
                    if j >= nb or (i < na and i * nb <= j * na * f):
                        out.append(a[i]); i += 1
                    else:
                        out.append(b[j]); j += 1
                return out

            # ---- schedule -----------------------------------------------
            # w0: qk(0) + v(0)           (wave 0 data only)
            # w1: qk(1)         x attn(0)
            # w2: qk(2) + v(1,2) x attn(1)
            # w3: qk(3) + v(3)   x attn(2)
            # w4: attn(3)
            for t in qk_thunks(0) + v_single_thunks(0):
                t()
            for t in weave(qk_thunks(1), attn_thunks(0)):
                t()
            for t in weave(v_pair_thunks(1, 2) + qk_thunks(2),
                           attn_thunks(1)):
                t()
            for t in weave(v_single_thunks(3) + qk_thunks(3),
                           attn_thunks(2)):
                t()
            for t in attn_thunks(3):
                t()

    nc.compile()
    return nc


def prepare_in_maps(x, wq, bq, wk, bk, wv, bv):
    f16 = np.float16
    f32 = np.float32
    sc = f32(C) ** -0.5
    xT = np.asarray(x).astype(f16).transpose(0, 2, 1)          # [B, C, T]
    # wave-major layout: xw[b, w, p, c*512+t] = xT[b, c*128+p, w*512+t]
    xw = np.ascontiguousarray(
        xT.reshape(B, N_KC, 128, N_JQ, TQ).transpose(0, 3, 2, 1, 4)
        .reshape(B, N_JQ, 128, N_KC * TQ))

    wqk = np.concatenate([np.asarray(wq) * sc, np.asarray(wk)], 1).astype(f16)
    wqk = np.ascontiguousarray(
        wqk.reshape(N_KC, 128, 128).transpose(1, 0, 2)).reshape(128, 1024)
    wv_c = np.asarray(wv).astype(f16)
    wv_c = np.ascontiguousarray(
        wv_c.reshape(N_KC, 128, H).transpose(1, 0, 2)).reshape(128, 512)
    ident = np.concatenate([np.eye(H), np.eye(H)], 0).astype(f16)
    r_i = np.arange(128)[:, None]
    c_i = np.arange(128)[None, :]
    tri = (c_i >= r_i).astype(f16)
    cst = np.ascontiguousarray(
        np.concatenate([wqk, wv_c, ident, tri], 1))
    assert cst.shape == (128, CST_W)

    bqk = np.concatenate([np.asarray(bq) * sc, np.asarray(bk)]) \
        .astype(f32).reshape(128, 1)
    bv2 = np.concatenate([np.asarray(bv), np.asarray(bv)]) \
        .astype(f32).reshape(128, 1)
    shared = {"cst": cst, "bqk": bqk, "bv2": bv2}
    return [{"xw": np.ascontiguousarray(xw[b]), **shared} for b in range(B)]


def postprocess(ys):
    out = np.empty((B, T, H), np.float32)
    for b, y in enumerate(ys):
        yf = y.astype(np.float32)
        out[b] = (yf[:H] / yf[H:H + 1]).T
    return out


def kernel(**inputs):
    global _CACHED_NC
    if _CACHED_NC is None:
        _CACHED_NC = build_program(reps=1)
    nc = _CACHED_NC
    in_maps = prepare_in_maps(
        inputs["x"], inputs["wq"], inputs["bq"], inputs["wk"],
        inputs["bk"], inputs["wv"], inputs["bv"])
    res = run_bass_kernel_spmd(nc, in_maps, core_ids=list(range(N_CORES)))
    return postprocess([r["y"] for r in res.results])


if __name__ == "__main__":
    rng = np.random.default_rng(0)
    demo = {
        "x": rng.standard_normal((B, T, C), dtype=np.float32),
        "wq": rng.standard_normal((C, H), dtype=np.float32) * 0.02,
        "bq": rng.standard_normal((H,), dtype=np.float32) * 0.02,
        "wk": rng.standard_normal((C, H), dtype=np.float32) * 0.02,
        "bk": rng.standard_normal((H,), dtype=np.float32) * 0.02,
        "wv": rng.standard_normal((C, H), dtype=np.float32) * 0.02,
        "bv": rng.standard_normal((H,), dtype=np.float32) * 0.02,
    }
    out = kernel(**demo)
    print("kernel output:", out.shape, out.dtype)
